# revision 42
# baseline (speedup 1.0000x reference)
"""Trainium2 Bass kernel for nn_Addparam_25701084299720 (retrieval_knn).

Computes, for N=4096 query points against V=16384 voxels:
  - 8-NN of each query (exact fp32 rescore of candidates from top-16
    segments-of-16 ranked by fp16 segment-min distance)
  - mean normal of the 8-NN, cosine-threshold mask vs all voxel normals
  - score_sum = sum_v mask * score_v/d_a * exp(-d_b*dist), score_num = sum mask
  - field = score_sum/max(score_num,1) masked by score_num!=0

Sharding: data-parallel over queries, 512 per core across 8 cores.
Voxel tables replicated.

Per-core algorithm (per 128-query tile):
  A. mmA (bf16x2 split, K=13): psum = 2 x.p - |p|^2 - |x|^2 =
     -(dist^2) to ~2e-5 (needed: d8 ~ 0.025 so one-level 16-bit inputs
     garble the NN ranking; hw float32r is only ~2e-4). ACT sqrt
     psum->Dch (fp16); DVE segment-min -> segsel ranking; ACT exp ->
     E (bf16); E *= score/d_a broadcast (DVE + GpSimd split).
  B. split in halves to hide the serial-gather latency: top-8 segments
     of segsel half 0 (DVE max/max_index) + 8 indirect-DMA gathers of
     packed 384B (p,n) segment blocks issue mid-A (after u==7); half 1
     at tile end -> exact fp32 diff-formulation rescore of 256
     candidates (matches the reference's rounding) -> top-8 -> xn =
     sum of their normals -> lC = bf16x2 rows of [xn, -0.8|xn|] via a
     DRAM round-trip transpose landing at partition base 32.
  C. mmC (bf16x2, K=12): margin = xn.nv - 0.8|xn||nv| per chunk-PAIR
     ([P,1024] psum; psC and the then-idle psA pools alternate in the
     exp window/tail): ss = DVE STT (margin>0)*E accum; cnt = ACT
     Sign+accum (Sign lives in every act table set; sqrt/exp batches
     are dependency-chained across tiles so the scheduler cannot
     interleave them and thrash table loads); cnt = (sum_sign + V)/2.
  D. field = ss/max(cnt,1) * (cnt>0).
"""
import sys

sys.path.insert(0, "/opt/trn_rl_repo")

import numpy as np
import ml_dtypes

N_CORES = 8
N = 4096
V = 16384
NQ = N // N_CORES          # 512 queries per core
P = 128                    # partitions
NT = NQ // P               # 4 query tiles per core
VCH = 512                  # voxel chunk (free dim per matmul)
NCH = V // VCH             # 32 chunks
SEGW = 16                  # voxels per segment
NSEG = V // SEGW           # 1024 segments
SPC = VCH // SEGW          # 32 segments per chunk
NCSEG = 16                 # candidate segments (8 per half)
NCAND = NCSEG * SEGW       # 256 candidate voxels
PKW = 6                    # packed floats per voxel in gather rows

# C-phase runs per chunk-PAIR ([P,1024] psum) to halve per-op overhead
# (decode + accumulator reads). GpSimd can neither read PSUM nor run
# TensorScalarPtr, so per pair: ss = DVE STT (pC>0)*E accum, cnt = ACT
# Sign+accum (sign lives in every act table set -> no table thrash).
NPAIR = NCH // 2               # 16 C pairs per tile
CNT_DVE = frozenset()  # pairs whose cnt runs on DVE (ACT won on hw)
ESCALE_GP = frozenset((0, 2, 4, 6))  # E*=score' slices on gpsimd

BF = ml_dtypes.bfloat16
F16 = np.float16
F32 = np.float32

_prog_cache = {}


def _build_program(neg_db: float, hw: bool = True):
    import concourse.bass as bass
    import concourse.mybir as mybir
    from concourse.tile import TileContext

    nc = bass.Bass()
    dt = mybir.dt
    AF = mybir.ActivationFunctionType
    OP = mybir.AluOpType

    lA_d = nc.declare_dram_parameter("lA", [13, NQ], dt.bfloat16,
                                     isOutput=False)
    tbl_d = nc.declare_dram_parameter("tbl", [44, V], dt.bfloat16,
                                      isOutput=False)
    pk_d = nc.declare_dram_parameter("pk", [NSEG, SEGW * PKW], dt.float32,
                                     isOutput=False)
    scp_d = nc.declare_dram_parameter("scp", [V], dt.bfloat16, isOutput=False)
    xq_d = nc.declare_dram_parameter("xq", [NQ, 3], dt.float32, isOutput=False)
    of_d = nc.declare_dram_parameter("of", [NQ], dt.float32, isOutput=True)
    on_d = nc.declare_dram_parameter("on", [NQ], dt.float32, isOutput=True)

    ts = bass.ts
    from concourse.tile_rust import add_dep_helper

    def act(*args, **kwargs):
        return nc.scalar.activation(*args, **kwargs)

    with TileContext(nc) as tc:
        with (
            tc.tile_pool(name="const", bufs=1) as constp,
            tc.tile_pool(name="bigd", bufs=2) as bigp,
            tc.tile_pool(name="small1", bufs=1) as smp1,
            tc.tile_pool(name="jkV", bufs=2) as jkV,
            tc.tile_pool(name="jkG", bufs=2) as jkG,
            tc.tile_pool(name="jkA", bufs=2) as jkA,
            tc.tile_pool(name="dch", bufs=NCH // 4 + 1) as dchp,
            tc.tile_pool(name="small", bufs=2) as smp,
            tc.tile_pool(name="drs", bufs=2, space="DRAM") as drp,
            tc.tile_pool(name="psA", bufs=2, space="PSUM") as psA,
            tc.tile_pool(name="psC", bufs=2, space="PSUM") as psC,
        ):
            lA = constp.tile([13, NQ], dt.bfloat16)
            tbl = constp.tile([44, V], dt.bfloat16)
            rA = tbl[0:13, :]
            rC = tbl[32:44, :]
            scbc = constp.tile([P, V], dt.bfloat16)
            eps4 = constp.tile([P, 1], dt.float32)
            nc.vector.memset(eps4[:], 4e-4)
            nc.sync.dma_start(lA[:], lA_d[:])
            Q4 = V // 4
            nc.sync.dma_start(tbl[:, 0:Q4], tbl_d[:, 0:Q4])
            nc.sync.dma_start(tbl[:, Q4:2 * Q4], tbl_d[:, Q4:2 * Q4])
            nc.scalar.dma_start(tbl[:, 2 * Q4:3 * Q4], tbl_d[:, 2 * Q4:3 * Q4])
            nc.scalar.dma_start(tbl[:, 3 * Q4:V], tbl_d[:, 3 * Q4:V])
            nc.scalar.dma_start(
                scbc[:],
                scp_d[:].rearrange("(o v) -> o v", o=1).to_broadcast([P, V]),
            )

            # ---------------- pipelined phase emission ----------------
            actchain = {"last": None}

            def chain_act(st, inst):
                """Bind the scheduler to the emitted sqrt/exp batch order —
                across tiles too, else it interleaves tile i+1's sqrt batch
                with tile i's exp batch and pays a table load per op (Sign
                needs no chaining: it lives in every table set)."""
                if actchain["last"] is not None:
                    add_dep_helper(inst.ins, actchain["last"].ins, sync=True,
                                   reason="ACT batch order")
                actchain["last"] = inst
                return inst

            def emit_exp_batch(E, dchs, k0, k1, st):
                """Exp over [P,2048] Dch tiles; interleave prev C chunks and
                the E*=score' scaling per covered slice."""
                SL = 4 * VCH
                for k in range(k0, k1):
                    chain_act(st, act(E[:, ts(k, SL)], dchs[k][:],
                                      AF.Exp, scale=neg_db))
                    if st["cprev"] is not None and st["cj"] < NPAIR:
                        emit_C_pairs(st["cprev"], st["cj"], st["cj"] + 1, st,
                                     alt=True)
                        st["cj"] += 1
                    eng = nc.gpsimd if k in ESCALE_GP else nc.vector
                    eng.tensor_tensor(
                        E[:, ts(k, SL)], E[:, ts(k, SL)],
                        scbc[:, ts(k, SL)], OP.mult,
                    )

            def emit_A(i, cprev):
                """Phase A of tile i, with tile i-1's C-chunks interleaved."""
                E = bigp.tile([P, V], dt.bfloat16, tag="E")
                segsel = smp.tile([P, NSEG], dt.float16, tag="segsel")
                xqt = smp.tile([P, 3], dt.float32, tag="xqt")
                nc.sync.dma_start(xqt[:], xq_d[ts(i, P), :])
                dchs = []
                st = {"cprev": cprev, "cj": 0, "lact": None}
                a_st = {"i": i, "E": E, "segsel": segsel, "xqt": xqt}
                for u in range(NCH // 2):
                    pA = psA.tile([P, 2 * VCH], dt.float32, tag="pA")
                    for half in range(2):
                        nc.tensor.matmul(
                            pA[:, ts(half, VCH)], lA[:, ts(i, P)],
                            rA[:, ts(2 * u + half, VCH)],
                            start=True, stop=True,
                        )
                    if u % 2 == 0:
                        Dch = dchp.tile([P, 4 * VCH], dt.float16, tag="Dch")
                        dchs.append(Dch)
                    Dch = dchs[-1]
                    half = u % 2
                    chain_act(st, act(
                        Dch[:, ts(half, 2 * VCH)], pA[:], AF.Sqrt,
                        bias=eps4[:, 0:1], scale=-1.0,
                    ))
                    if half == 1:
                        nc.vector.tensor_reduce(
                            segsel[:, ts(u // 2, 4 * SPC)],
                            Dch[:].rearrange("p (s w) -> p s w", w=SEGW),
                            axis=mybir.AxisListType.X, op=OP.min, negate=True,
                        )
                    if u == 7:
                        emit_B_half0(a_st)
                    if cprev is not None:
                        if u == 5:
                            emit_finishB(cprev, st)
                        if u >= 6 and st["cj"] < NPAIR:
                            emit_C_pairs(cprev, st["cj"], st["cj"] + 1, st)
                            st["cj"] += 1
                emit_exp_batch(E, dchs, 0, NCH // 4, st)
                return a_st

            def emit_B_half0(a):
                """First-half candidate selection: segsel[:, 0:512] is
                complete after u==7, so the serial indirect gathers (the
                longest B-latency item) start mid-A and overlap u8-15."""
                segsel, xqt = a["segsel"], a["xqt"]
                m8s = smp.tile([P, NCSEG], dt.float16, tag="m8s")
                sidx = smp.tile([P, NCSEG], dt.uint32, tag="sidx")
                HS = NSEG // 2
                nc.vector.max(m8s[:, 0:8], segsel[:, 0:HS])
                nc.vector.max_index(sidx[:, 0:8], m8s[:, 0:8], segsel[:, 0:HS])
                pkg = smp1.tile([P, NCSEG, SEGW * PKW], dt.float32, tag="pkg")
                # hardware indirect DMA consumes ONE index per partition:
                # one gather per candidate segment
                for g in range(8):
                    nc.gpsimd.indirect_dma_start(
                        out=pkg[:, g, :], out_offset=None,
                        in_=pk_d[:],
                        in_offset=bass.IndirectOffsetOnAxis(
                            ap=sidx[:, g:g + 1], axis=0),
                    )
                pkv = pkg[:].rearrange("p s (w c) -> p s w c", c=PKW)
                HC = NCAND // 2
                df0 = smp1.tile([P, NCAND], dt.float32, tag="df0")
                df1 = smp1.tile([P, NCAND], dt.float32, tag="df1")
                df2 = smp1.tile([P, NCAND], dt.float32, tag="df2")
                sq0 = smp1.tile([P, NCAND], dt.float32, tag="sq0")
                sq1 = smp1.tile([P, NCAND], dt.float32, tag="sq1")
                sq2 = smp1.tile([P, NCAND], dt.float32, tag="sq2")
                for c in range(3):
                    df = (df0, df1, df2)[c]
                    sq = (sq0, sq1, sq2)[c]
                    # fl(p - x) then fl(square): same rounding as reference
                    nc.vector.tensor_scalar(
                        df[:, 0:HC], pkv[:, 0:8, :, c], xqt[:, c:c + 1], None,
                        OP.subtract,
                    )
                    nc.gpsimd.tensor_tensor(sq[:, 0:HC], df[:, 0:HC],
                                            df[:, 0:HC], OP.mult)
                a.update(m8s=m8s, sidx=sidx, pkg=pkg, dfs=(df0, df1, df2),
                         sqs=(sq0, sq1, sq2))

            def emit_B(a):
                segsel, xqt = a["segsel"], a["xqt"]
                m8s, sidx, pkg = a["m8s"], a["sidx"], a["pkg"]
                df0, df1, df2 = a["dfs"]
                sq0, sq1, sq2 = a["sqs"]
                HS = NSEG // 2
                HC = NCAND // 2
                nc.vector.max(m8s[:, 8:16], segsel[:, HS:NSEG])
                nc.vector.max_index(sidx[:, 8:16], m8s[:, 8:16],
                                    segsel[:, HS:NSEG])
                nc.vector.tensor_scalar(
                    sidx[:, 8:16], sidx[:, 8:16], HS, None, OP.add
                )
                for g in range(8, NCSEG):
                    nc.gpsimd.indirect_dma_start(
                        out=pkg[:, g, :], out_offset=None,
                        in_=pk_d[:],
                        in_offset=bass.IndirectOffsetOnAxis(
                            ap=sidx[:, g:g + 1], axis=0),
                    )
                pkv = pkg[:].rearrange("p s (w c) -> p s w c", c=PKW)
                for c in range(3):
                    df = (df0, df1, df2)[c]
                    sq = (sq0, sq1, sq2)[c]
                    nc.vector.tensor_scalar(
                        df[:, HC:], pkv[:, 8:16, :, c], xqt[:, c:c + 1], None,
                        OP.subtract,
                    )
                    nc.gpsimd.tensor_tensor(sq[:, HC:], df[:, HC:],
                                            df[:, HC:], OP.mult)
                # exd2 -> sq0 (in place), negk -> sq1, selx -> sq2
                nc.gpsimd.tensor_tensor(sq0[:], sq0[:], sq1[:], OP.add)
                nc.gpsimd.tensor_tensor(sq0[:], sq0[:], sq2[:], OP.add)
                nc.vector.tensor_scalar(sq1[:], sq0[:], -1.0, None, OP.mult)
                m8x = smp.tile([P, 8], dt.float32, tag="m8x")
                nc.vector.max(m8x[:], sq1[:])
                nc.vector.tensor_scalar(
                    sq2[:], sq1[:], m8x[:, 7:8], None, OP.is_ge
                )
                xa4 = smp.tile([P, 4], dt.float32, tag="xa4")
                for c in range(3):
                    nc.vector.scalar_tensor_tensor(
                        out=(df1, df0, df0)[c][:], in0=sq2[:], scalar=1.0,
                        in1=pkv[:, :, :, 3 + c],
                        op0=OP.mult, op1=OP.mult,
                        accum_out=xa4[:, c:c + 1],
                    )
                a2 = smp.tile([P, 1], dt.float32, tag="a2")
                nc.vector.scalar_tensor_tensor(
                    out=df2[:, 0:3], in0=xa4[:, 0:3], scalar=1.0,
                    in1=xa4[:, 0:3], op0=OP.mult, op1=OP.mult,
                    accum_out=a2[:],
                )
                return {"i": a["i"], "E": a["E"], "xa4": xa4, "a2": a2}

            def emit_finishB(b, st=None):
                # NOTE: xnn is deliberately NOT in the ACT chain: it depends
                # on the previous tile's B rescore, and chaining it would
                # stall the whole sqrt batch behind that. Unchained it lands
                # mid-sqrt-batch where the sqrt table is already resident.
                xa4 = b["xa4"]
                xnn = smp.tile([P, 1], dt.float32, tag="xnn")
                act(xnn[:], b["a2"][:], AF.Sqrt)
                # bf16x2 lC rows matching rC: [xah x2, xal, ch, ch, cl];
                # packed [P,32] then one XBAR DMA transpose lands them at
                # partition base 32 to match rC's base in tbl
                lCt = smp.tile([P, 12], dt.bfloat16, tag="lCt")
                tmp3 = smp.tile([P, 3], dt.float32, tag="tmp3")
                cc1 = smp.tile([P, 1], dt.float32, tag="cc1")
                nc.vector.tensor_copy(lCt[:, 0:3], xa4[:, 0:3])
                nc.vector.tensor_copy(lCt[:, 3:6], lCt[:, 0:3])
                nc.vector.tensor_copy(tmp3[:], lCt[:, 0:3])
                nc.vector.tensor_tensor(tmp3[:], xa4[:, 0:3], tmp3[:],
                                        OP.subtract)
                nc.vector.tensor_copy(lCt[:, 6:9], tmp3[:])
                nc.vector.tensor_scalar(cc1[:], xnn[:], -0.8, None, OP.mult)
                nc.vector.tensor_copy(lCt[:, 9:10], cc1[:])
                nc.vector.tensor_copy(lCt[:, 10:11], lCt[:, 9:10])
                nc.vector.tensor_copy(tmp3[:, 0:1], lCt[:, 9:10])
                nc.vector.tensor_tensor(tmp3[:, 0:1], cc1[:], tmp3[:, 0:1],
                                        OP.subtract)
                nc.vector.tensor_copy(lCt[:, 11:12], tmp3[:, 0:1])
                # DRAM round trip: store [P,12] then load back with the
                # axes swapped; lands at partition base 32 to match rC
                xad = drp.tile([P, 12], dt.bfloat16, tag="xad")
                nc.sync.dma_start(xad[:], lCt[:])
                lCp = smp.tile([44, P], dt.bfloat16, tag="lCp")
                lC = lCp[32:44, :]
                nc.sync.dma_start(lC, xad[:].rearrange("a b -> b a"))
                b["lC"] = lC
                ssV = smp.tile([P, NPAIR], dt.float32, tag="ssV")
                sgn32 = smp.tile([P, NPAIR - len(CNT_DVE)], dt.float32,
                                 tag="sgn32")
                cntV = (smp.tile([P, len(CNT_DVE)], dt.float32, tag="cntV")
                        if CNT_DVE else None)
                b.update(ssV=ssV, sgn32=sgn32, cntV=cntV, jsg=0, jc=0)

            def emit_C_pairs(b, pj0, pj1, st=None, alt=False):
                lC, E = b["lC"], b["E"]
                for pj in range(pj0, pj1):
                    # during the exp window and the tail, psA sits idle (the
                    # last sqrt released it): alternate pools to double the
                    # psum drain depth
                    pool = psA if (alt and pj % 2 == 1) else psC
                    pC = pool.tile([P, 2 * VCH], dt.float32,
                                   tag="pA" if pool is psA else "pC")
                    for half in range(2):
                        nc.tensor.matmul(
                            pC[:, ts(half, VCH)], lC,
                            rC[:, ts(2 * pj + half, VCH)],
                            start=True, stop=True,
                        )
                    jnk = jkV.tile([P, 2 * VCH], dt.bfloat16, tag="jnkv")
                    nc.vector.scalar_tensor_tensor(
                        out=jnk[:], in0=pC[:], scalar=0.0,
                        in1=E[:, ts(pj, 2 * VCH)],
                        op0=OP.is_gt, op1=OP.mult,
                        accum_out=b["ssV"][:, pj:pj + 1],
                    )
                    if pj in CNT_DVE:
                        jnk2 = jkV.tile([P, 2 * VCH], dt.bfloat16, tag="jnkc")
                        nc.vector.tensor_scalar(
                            jnk2[:], pC[:], 0.0, None, OP.is_gt, OP.add,
                            accum_out=b["cntV"][:, b["jc"]:b["jc"] + 1],
                        )
                        b["jc"] += 1
                    else:
                        jnk2 = jkA.tile([P, 2 * VCH], dt.bfloat16, tag="jnka")
                        act(jnk2[:], pC[:], AF.Sign,
                            accum_out=b["sgn32"][:, b["jsg"]:b["jsg"] + 1])
                        b["jsg"] += 1

            def emit_D(b):
                i = b["i"]
                sst = smp.tile([P, 1], dt.float32, tag="sst")
                sgs = smp.tile([P, 1], dt.float32, tag="sgs")
                nc.vector.reduce_sum(sst[:], b["ssV"][:],
                                     axis=mybir.AxisListType.X)
                nc.vector.reduce_sum(sgs[:], b["sgn32"][:],
                                     axis=mybir.AxisListType.X)
                cntt = smp.tile([P, 1], dt.float32, tag="cntt")
                # cnt = cnt_dve + (sum_sign + 1024*n_sign_pairs)/2
                nc.vector.tensor_scalar(
                    cntt[:], sgs[:], 0.5,
                    float(VCH * (NPAIR - len(CNT_DVE))),
                    OP.mult, OP.add,
                )
                if b["cntV"] is not None:
                    cnv = smp.tile([P, 1], dt.float32, tag="cnv")
                    nc.vector.reduce_sum(cnv[:], b["cntV"][:],
                                         axis=mybir.AxisListType.X)
                    nc.vector.tensor_tensor(cntt[:], cntt[:], cnv[:], OP.add)
                nz = smp.tile([P, 1], dt.float32, tag="nz")
                nc.vector.tensor_scalar(nz[:], cntt[:], 0.5, None, OP.is_gt)
                cc = smp.tile([P, 1], dt.float32, tag="cc")
                nc.vector.tensor_scalar(cc[:], cntt[:], 1.0, None, OP.max)
                rec = smp.tile([P, 1], dt.float32, tag="rec")
                nc.vector.reciprocal(rec[:], cc[:])
                fld = smp.tile([P, 1], dt.float32, tag="fld")
                nc.vector.tensor_tensor(fld[:], sst[:], rec[:], OP.mult)
                nc.vector.tensor_tensor(fld[:], fld[:], nz[:], OP.mult)
                nc.sync.dma_start(of_d[ts(i, P)], fld[:])
                nc.sync.dma_start(on_d[ts(i, P)], nz[:])

            prev = None
            for i in range(NT):
                a = emit_A(i, prev)
                if prev is not None:
                    emit_D(prev)
                prev = emit_B(a)
            emit_finishB(prev)
            emit_C_pairs(prev, 0, NPAIR, alt=True)
            emit_D(prev)

    if hw:
        _split_multiwaits(nc)
    return nc


def _split_multiwaits(nc):
    """This toolchain's walrus accepts at most ONE sync wait per
    instruction (setupSyncWait<...> hard-errors otherwise). Tile attaches
    all required waits to the consuming instruction, so split every
    extra wait into a standalone EventSemaphore on the same engine queue
    right before the instruction (the raw-Bass wait_ge pattern)."""
    import concourse.mybir as mybir

    n = 0
    for bb in nc.main_func.blocks:
        insts = bb.instructions
        out = []
        for inst in insts:
            si = inst.sync_info
            if si is not None and len(si.on_wait) > 1:
                waits = list(si.on_wait)
                for w in waits[:-1]:
                    ev = mybir.InstEventSemaphore(name=f"W-split-{n}")
                    n += 1
                    ev.engine = inst.engine
                    ev.debug = inst.debug
                    ev.sync_info = mybir.SyncInfo(on_wait=[w], on_update=[])
                    out.append(ev)
                inst.sync_info = mybir.SyncInfo(
                    on_wait=[waits[-1]], on_update=list(si.on_update)
                )
            out.append(inst)
        bb.instructions = out


def _prep_inputs(x_world, voxel_point, voxel_normal, score, d_a, d_b):
    """Host-side prep: per-core in_maps for the SPMD program."""
    x = np.ascontiguousarray(x_world[:, 0, :], dtype=F32)          # [N,3]
    p = np.ascontiguousarray(voxel_point[0, :, :3], dtype=F32)     # [V,3]
    nrm = np.ascontiguousarray(voxel_normal, dtype=F32)            # [V,3]
    sc = np.asarray(score, dtype=F32)
    da = float(np.asarray(d_a).reshape(-1)[0])
    db = float(np.asarray(d_b).reshape(-1)[0])

    def s2(a):
        """bf16x2 split: hi + lo as float32."""
        h = a.astype(BF).astype(F32)
        return h, (a - h).astype(F32)

    # tbl rows 0-12 (rA): per coord [ph, pl, ph], then [1, 1, p2h, p2l]
    # pairing lA rows [xh, xh, xl]*3, [x2h, x2l, 1, 1]:
    #   psum = 2x.p - x2 - p2 = -(dist^2) to ~2e-5
    # tbl rows 32-43 (rC): [nh(3), nl(3), nh(3), bh, bl, bh]
    p2h, p2l = s2(-(p * p).sum(1, dtype=F32))
    b = np.sqrt((nrm * nrm).sum(1, dtype=F32)).astype(F32)
    nh, nl = s2(nrm)
    bh, bl = s2(b)
    tbl = np.zeros((44, V), F32)
    for i in range(3):
        ph, pl = s2(p[:, i])
        tbl[3 * i + 0] = ph
        tbl[3 * i + 1] = pl
        tbl[3 * i + 2] = ph
    tbl[9] = 1.0
    tbl[10] = 1.0
    tbl[11] = p2h
    tbl[12] = p2l
    tbl[32:35] = nh.T
    tbl[35:38] = nl.T
    tbl[38:41] = nh.T
    tbl[41] = bh
    tbl[42] = bl
    tbl[43] = bh
    tbl = tbl.astype(BF)

    pk = np.zeros((V, PKW), F32)
    pk[:, 0:3] = p
    pk[:, 3:6] = nrm
    pk16 = np.ascontiguousarray(pk.reshape(NSEG, SEGW * PKW))
    scp = (sc * (1.0 / da)).astype(F32).astype(BF)

    in_maps = []
    for cid in range(N_CORES):
        sl = slice(cid * NQ, (cid + 1) * NQ)
        xc = x[sl]                                                  # [NQ,3]
        x2h, x2l = s2(-(xc * xc).sum(1, dtype=F32))
        lA = np.zeros((13, NQ), F32)
        for i in range(3):
            xh, xl = s2(2.0 * xc[:, i])
            lA[3 * i + 0] = xh
            lA[3 * i + 1] = xh
            lA[3 * i + 2] = xl
        lA[9] = x2h
        lA[10] = x2l
        lA[11] = 1.0
        lA[12] = 1.0
        in_maps.append({
            "lA": lA.astype(BF), "tbl": tbl, "pk": pk16, "scp": scp,
            "xq": xc,
        })
    return in_maps, db


def _get_runner(nc):
    """Build (once) a jitted 8-core SPMD runner for the program."""
    import jax
    from jax.sharding import Mesh, PartitionSpec, NamedSharding
    try:
        from jax.experimental.shard_map import shard_map
    except Exception:
        from jax.shard_map import shard_map
    from concourse import bass2jax
    import concourse.mybir as mybir

    bass2jax.install_neuronx_cc_hook()
    pname = nc.partition_id_tensor.name if nc.partition_id_tensor else None
    in_names, out_names, out_avals, zero_outs = [], [], [], []
    for alloc in nc.m.functions[0].allocations:
        if not isinstance(alloc, mybir.MemoryLocationSet):
            continue
        name = alloc.memorylocations[0].name
        if alloc.kind == "ExternalInput":
            if name != pname:
                in_names.append(name)
        elif alloc.kind == "ExternalOutput":
            shape = tuple(alloc.tensor_shape)
            dtype = mybir.dt.np(alloc.dtype)
            out_names.append(name)
            out_avals.append(jax.core.ShapedArray(shape, dtype))
            zero_outs.append(np.zeros(shape, dtype))
    all_names = list(in_names) + list(out_names) + ([pname] if pname else [])

    def _body(*args):
        operands = list(args)
        if pname:
            operands.append(bass2jax.partition_id_tensor())
        return tuple(bass2jax._bass_exec_p.bind(
            *operands, out_avals=tuple(out_avals), in_names=tuple(all_names),
            out_names=tuple(out_names), lowering_input_output_aliases=(),
            sim_require_finite=True, sim_require_nnan=True, nc=nc))

    devices = jax.devices()[:N_CORES]
    mesh = Mesh(np.asarray(devices), ("core",))
    nin = len(in_names) + len(out_names)
    fn = jax.jit(shard_map(
        _body, mesh=mesh, in_specs=(PartitionSpec("core"),) * nin,
        out_specs=(PartitionSpec("core"),) * len(out_names),
        check_rep=False), keep_unused=True)
    sharding = NamedSharding(mesh, PartitionSpec("core"))

    def run(in_maps):
        concat = [np.concatenate([np.asarray(in_maps[c][nm])
                                  for c in range(N_CORES)], axis=0)
                  for nm in in_names]
        concat += [np.concatenate([z] * N_CORES, axis=0) for z in zero_outs]
        import jax as _j
        dev = [_j.device_put(a, sharding) for a in concat]
        outs = fn(*dev)
        o = {nm: np.asarray(outs[i]) for i, nm in enumerate(out_names)}
        return o

    return run


def kernel(**inputs):
    in_maps, db = _prep_inputs(
        inputs["x_world"], inputs["voxel_point"], inputs["voxel_normal"],
        inputs["score"], inputs["d_a"], inputs["d_b"],
    )
    key = ("prog", db)
    if key not in _prog_cache:
        _prog_cache[key] = _build_program(-db)
    nc = _prog_cache[key]

    try:
        rkey = ("runner", db)
        if rkey not in _prog_cache:
            _prog_cache[rkey] = _get_runner(nc)
        o = _prog_cache[rkey](in_maps)
        field = o["of"].reshape(N_CORES, NQ).reshape(-1)
        nzf = o["on"].reshape(-1)
    except Exception:
        from concourse.bass_utils import run_bass_kernel_spmd
        res = run_bass_kernel_spmd(nc, in_maps, list(range(N_CORES))).results
        field = np.concatenate([np.asarray(r["of"]).reshape(-1) for r in res])
        nzf = np.concatenate([np.asarray(r["on"]).reshape(-1) for r in res])
    return field.astype(F32), (nzf > 0.5)


# revision 43
# speedup vs baseline: 1.8048x; 1.8048x over previous
"""Trainium2 Bass kernel for nn_Addparam_25701084299720 (retrieval_knn).

Computes, for N=4096 query points against V=16384 voxels:
  - 8-NN of each query (exact fp32 rescore of candidates from top-16
    segments-of-16 ranked by fp16 segment-min distance)
  - mean normal of the 8-NN, cosine-threshold mask vs all voxel normals
  - score_sum = sum_v mask * score_v/d_a * exp(-d_b*dist), score_num = sum mask
  - field = score_sum/max(score_num,1) masked by score_num!=0

Sharding: data-parallel over queries, 512 per core across 8 cores.
Voxel tables replicated.

Per-core algorithm (per 128-query tile):
  A. mmA (bf16x2 split, K=13): psum = 2 x.p - |p|^2 - |x|^2 =
     -(dist^2) to ~2e-5 (needed: d8 ~ 0.025 so one-level 16-bit inputs
     garble the NN ranking; hw float32r is only ~2e-4). ACT sqrt
     psum->Dch (fp16); DVE segment-min -> segsel ranking; ACT exp ->
     E (bf16); E *= score/d_a broadcast (DVE + GpSimd split).
  B. split in halves to hide the serial-gather latency: top-8 segments
     of segsel half 0 (DVE max/max_index) + 8 indirect-DMA gathers of
     packed 384B (p,n) segment blocks issue mid-A (after u==7); half 1
     at tile end -> exact fp32 diff-formulation rescore of 256
     candidates (matches the reference's rounding) -> top-8 -> xn =
     sum of their normals -> lC = bf16x2 rows of [xn, -0.8|xn|] via a
     DRAM round-trip transpose landing at partition base 32.
  C. mmC (bf16x2, K=12): margin = xn.nv - 0.8|xn||nv| per chunk-PAIR
     ([P,1024] psum; psC and the then-idle psA pools alternate in the
     exp window/tail): ss = DVE STT (margin>0)*E accum; cnt = ACT
     Sign+accum (Sign lives in every act table set; sqrt/exp batches
     are dependency-chained across tiles so the scheduler cannot
     interleave them and thrash table loads); cnt = (sum_sign + V)/2.
  D. field = ss/max(cnt,1) * (cnt>0).
"""
import sys

sys.path.insert(0, "/opt/trn_rl_repo")

import numpy as np
import ml_dtypes

N_CORES = 8
N = 4096
V = 16384
NQ = N // N_CORES          # 512 queries per core
P = 128                    # partitions
NT = NQ // P               # 4 query tiles per core
VCH = 512                  # voxel chunk (free dim per matmul)
NCH = V // VCH             # 32 chunks
SEGW = 16                  # voxels per segment
NSEG = V // SEGW           # 1024 segments
SPC = VCH // SEGW          # 32 segments per chunk
NCSEG = 16                 # candidate segments (8 per half)
NCAND = NCSEG * SEGW       # 256 candidate voxels
PKW = 6                    # packed floats per voxel in gather rows

# C-phase runs per chunk-PAIR ([P,1024] psum) to halve per-op overhead
# (decode + accumulator reads). GpSimd can neither read PSUM nor run
# TensorScalarPtr, so per pair: ss = DVE STT (pC>0)*E accum, cnt = ACT
# Sign+accum (sign lives in every act table set -> no table thrash).
NPAIR = NCH // 2               # 16 C pairs per tile
CNT_DVE = frozenset()  # pairs whose cnt runs on DVE (ACT won on hw)
ESCALE_GP = frozenset((0, 2, 4, 6))  # E*=score' slices on gpsimd

BF = ml_dtypes.bfloat16
F16 = np.float16
F32 = np.float32

_prog_cache = {}


def _build_program(neg_db: float, hw: bool = True):
    import concourse.bass as bass
    import concourse.mybir as mybir
    from concourse.tile import TileContext

    nc = bass.Bass()
    dt = mybir.dt
    AF = mybir.ActivationFunctionType
    OP = mybir.AluOpType

    lA_d = nc.declare_dram_parameter("lA", [13, NQ], dt.bfloat16,
                                     isOutput=False)
    tbl_d = nc.declare_dram_parameter("tbl", [44, V], dt.bfloat16,
                                      isOutput=False)
    pk_d = nc.declare_dram_parameter("pk", [NSEG, SEGW * PKW], dt.float32,
                                     isOutput=False)
    scp_d = nc.declare_dram_parameter("scp", [V], dt.bfloat16, isOutput=False)
    xq_d = nc.declare_dram_parameter("xq", [NQ, 3], dt.float32, isOutput=False)
    of_d = nc.declare_dram_parameter("of", [NQ], dt.float32, isOutput=True)
    on_d = nc.declare_dram_parameter("on", [NQ], dt.float32, isOutput=True)

    ts = bass.ts
    from concourse.tile_rust import add_dep_helper

    def act(*args, **kwargs):
        return nc.scalar.activation(*args, **kwargs)

    with TileContext(nc) as tc:
        with (
            tc.tile_pool(name="const", bufs=1) as constp,
            tc.tile_pool(name="bigd", bufs=2) as bigp,
            tc.tile_pool(name="small1", bufs=1) as smp1,
            tc.tile_pool(name="jkV", bufs=2) as jkV,
            tc.tile_pool(name="jkG", bufs=2) as jkG,
            tc.tile_pool(name="jkA", bufs=2) as jkA,
            tc.tile_pool(name="dch", bufs=NCH // 4 + 1) as dchp,
            tc.tile_pool(name="small", bufs=2) as smp,
            tc.tile_pool(name="drs", bufs=2, space="DRAM") as drp,
            tc.tile_pool(name="psA", bufs=2, space="PSUM") as psA,
            tc.tile_pool(name="psC", bufs=2, space="PSUM") as psC,
        ):
            lA = constp.tile([13, NQ], dt.bfloat16)
            tbl = constp.tile([44, V], dt.bfloat16)
            rA = tbl[0:13, :]
            rC = tbl[32:44, :]
            scbc = constp.tile([P, V], dt.bfloat16)
            eps4 = constp.tile([P, 1], dt.float32)
            nc.vector.memset(eps4[:], 4e-4)
            nc.sync.dma_start(lA[:], lA_d[:])
            Q4 = V // 4
            nc.sync.dma_start(tbl[:, 0:Q4], tbl_d[:, 0:Q4])
            nc.sync.dma_start(tbl[:, Q4:2 * Q4], tbl_d[:, Q4:2 * Q4])
            nc.scalar.dma_start(tbl[:, 2 * Q4:3 * Q4], tbl_d[:, 2 * Q4:3 * Q4])
            nc.scalar.dma_start(tbl[:, 3 * Q4:V], tbl_d[:, 3 * Q4:V])
            nc.scalar.dma_start(
                scbc[:],
                scp_d[:].rearrange("(o v) -> o v", o=1).to_broadcast([P, V]),
            )

            # ---------------- pipelined phase emission ----------------
            actchain = {"last": None}

            def chain_act(st, inst):
                """Bind the scheduler to the emitted sqrt/exp batch order —
                across tiles too, else it interleaves tile i+1's sqrt batch
                with tile i's exp batch and pays a table load per op (Sign
                needs no chaining: it lives in every table set)."""
                if actchain["last"] is not None:
                    add_dep_helper(inst.ins, actchain["last"].ins, sync=True,
                                   reason="ACT batch order")
                actchain["last"] = inst
                return inst

            def emit_exp_batch(E, dchs, k0, k1, st):
                """Exp over [P,2048] Dch tiles; interleave prev C chunks and
                the E*=score' scaling per covered slice."""
                SL = 4 * VCH
                for k in range(k0, k1):
                    chain_act(st, act(E[:, ts(k, SL)], dchs[k][:],
                                      AF.Exp, scale=neg_db))
                    if st["cprev"] is not None and st["cj"] < NPAIR:
                        emit_C_pairs(st["cprev"], st["cj"], st["cj"] + 1, st,
                                     alt=True)
                        st["cj"] += 1
                    eng = nc.gpsimd if k in ESCALE_GP else nc.vector
                    eng.tensor_tensor(
                        E[:, ts(k, SL)], E[:, ts(k, SL)],
                        scbc[:, ts(k, SL)], OP.mult,
                    )

            def emit_A(i, cprev):
                """Phase A of tile i, with tile i-1's C-chunks interleaved."""
                E = bigp.tile([P, V], dt.bfloat16, tag="E")
                segsel = smp.tile([P, NSEG], dt.float16, tag="segsel")
                xqt = smp.tile([P, 3], dt.float32, tag="xqt")
                nc.sync.dma_start(xqt[:], xq_d[ts(i, P), :])
                dchs = []
                st = {"cprev": cprev, "cj": 0, "lact": None}
                a_st = {"i": i, "E": E, "segsel": segsel, "xqt": xqt}
                for u in range(NCH // 2):
                    pA = psA.tile([P, 2 * VCH], dt.float32, tag="pA")
                    for half in range(2):
                        nc.tensor.matmul(
                            pA[:, ts(half, VCH)], lA[:, ts(i, P)],
                            rA[:, ts(2 * u + half, VCH)],
                            start=True, stop=True,
                        )
                    if u % 2 == 0:
                        Dch = dchp.tile([P, 4 * VCH], dt.float16, tag="Dch")
                        dchs.append(Dch)
                    Dch = dchs[-1]
                    half = u % 2
                    chain_act(st, act(
                        Dch[:, ts(half, 2 * VCH)], pA[:], AF.Sqrt,
                        bias=eps4[:, 0:1], scale=-1.0,
                    ))
                    if half == 1:
                        nc.vector.tensor_reduce(
                            segsel[:, ts(u // 2, 4 * SPC)],
                            Dch[:].rearrange("p (s w) -> p s w", w=SEGW),
                            axis=mybir.AxisListType.X, op=OP.min, negate=True,
                        )
                    if u == 7:
                        emit_B_half0(a_st)
                    if cprev is not None:
                        if u == 2:
                            emit_finishB(cprev, st)
                        if u >= 6 and st["cj"] < NPAIR:
                            emit_C_pairs(cprev, st["cj"], st["cj"] + 1, st)
                            st["cj"] += 1
                emit_exp_batch(E, dchs, 0, NCH // 4, st)
                return a_st

            def emit_B_half0(a):
                """First-half candidate selection: segsel[:, 0:512] is
                complete after u==7, so the serial indirect gathers (the
                longest B-latency item) start mid-A and overlap u8-15."""
                segsel, xqt = a["segsel"], a["xqt"]
                m8s = smp.tile([P, NCSEG], dt.float16, tag="m8s")
                sidx = smp.tile([P, NCSEG], dt.uint32, tag="sidx")
                HS = NSEG // 2
                nc.vector.max(m8s[:, 0:8], segsel[:, 0:HS])
                nc.vector.max_index(sidx[:, 0:8], m8s[:, 0:8], segsel[:, 0:HS])
                pkg = smp1.tile([P, NCSEG, SEGW * PKW], dt.float32, tag="pkg")
                # hardware indirect DMA consumes ONE index per partition:
                # one gather per candidate segment
                for g in range(8):
                    nc.gpsimd.indirect_dma_start(
                        out=pkg[:, g, :], out_offset=None,
                        in_=pk_d[:],
                        in_offset=bass.IndirectOffsetOnAxis(
                            ap=sidx[:, g:g + 1], axis=0),
                    )
                pkv = pkg[:].rearrange("p s (w c) -> p s w c", c=PKW)
                HC = NCAND // 2
                df0 = smp1.tile([P, NCAND], dt.float32, tag="df0")
                df1 = smp1.tile([P, NCAND], dt.float32, tag="df1")
                df2 = smp1.tile([P, NCAND], dt.float32, tag="df2")
                sq0 = smp1.tile([P, NCAND], dt.float32, tag="sq0")
                sq1 = smp1.tile([P, NCAND], dt.float32, tag="sq1")
                sq2 = smp1.tile([P, NCAND], dt.float32, tag="sq2")
                for c in range(3):
                    df = (df0, df1, df2)[c]
                    sq = (sq0, sq1, sq2)[c]
                    # fl(p - x) then fl(square): same rounding as reference
                    nc.vector.tensor_scalar(
                        df[:, 0:HC], pkv[:, 0:8, :, c], xqt[:, c:c + 1], None,
                        OP.subtract,
                    )
                    nc.gpsimd.tensor_tensor(sq[:, 0:HC], df[:, 0:HC],
                                            df[:, 0:HC], OP.mult)
                a.update(m8s=m8s, sidx=sidx, pkg=pkg, dfs=(df0, df1, df2),
                         sqs=(sq0, sq1, sq2))

            def emit_B(a):
                segsel, xqt = a["segsel"], a["xqt"]
                m8s, sidx, pkg = a["m8s"], a["sidx"], a["pkg"]
                df0, df1, df2 = a["dfs"]
                sq0, sq1, sq2 = a["sqs"]
                HS = NSEG // 2
                HC = NCAND // 2
                nc.vector.max(m8s[:, 8:16], segsel[:, HS:NSEG])
                nc.vector.max_index(sidx[:, 8:16], m8s[:, 8:16],
                                    segsel[:, HS:NSEG])
                nc.vector.tensor_scalar(
                    sidx[:, 8:16], sidx[:, 8:16], HS, None, OP.add
                )
                for g in range(8, NCSEG):
                    nc.gpsimd.indirect_dma_start(
                        out=pkg[:, g, :], out_offset=None,
                        in_=pk_d[:],
                        in_offset=bass.IndirectOffsetOnAxis(
                            ap=sidx[:, g:g + 1], axis=0),
                    )
                pkv = pkg[:].rearrange("p s (w c) -> p s w c", c=PKW)
                for c in range(3):
                    df = (df0, df1, df2)[c]
                    sq = (sq0, sq1, sq2)[c]
                    nc.vector.tensor_scalar(
                        df[:, HC:], pkv[:, 8:16, :, c], xqt[:, c:c + 1], None,
                        OP.subtract,
                    )
                    nc.gpsimd.tensor_tensor(sq[:, HC:], df[:, HC:],
                                            df[:, HC:], OP.mult)
                # exd2 -> sq0 (in place), negk -> sq1, selx -> sq2
                nc.gpsimd.tensor_tensor(sq0[:], sq0[:], sq1[:], OP.add)
                nc.gpsimd.tensor_tensor(sq0[:], sq0[:], sq2[:], OP.add)
                nc.vector.tensor_scalar(sq1[:], sq0[:], -1.0, None, OP.mult)
                m8x = smp.tile([P, 8], dt.float32, tag="m8x")
                nc.vector.max(m8x[:], sq1[:])
                nc.vector.tensor_scalar(
                    sq2[:], sq1[:], m8x[:, 7:8], None, OP.is_ge
                )
                xa4 = smp.tile([P, 4], dt.float32, tag="xa4")
                for c in range(3):
                    nc.vector.scalar_tensor_tensor(
                        out=(df1, df0, df0)[c][:], in0=sq2[:], scalar=1.0,
                        in1=pkv[:, :, :, 3 + c],
                        op0=OP.mult, op1=OP.mult,
                        accum_out=xa4[:, c:c + 1],
                    )
                a2 = smp.tile([P, 1], dt.float32, tag="a2")
                nc.vector.scalar_tensor_tensor(
                    out=df2[:, 0:3], in0=xa4[:, 0:3], scalar=1.0,
                    in1=xa4[:, 0:3], op0=OP.mult, op1=OP.mult,
                    accum_out=a2[:],
                )
                return {"i": a["i"], "E": a["E"], "xa4": xa4, "a2": a2}

            def emit_finishB(b, st=None):
                # NOTE: xnn is deliberately NOT in the ACT chain: it depends
                # on the previous tile's B rescore, and chaining it would
                # stall the whole sqrt batch behind that. Unchained it lands
                # mid-sqrt-batch where the sqrt table is already resident.
                xa4 = b["xa4"]
                xnn = smp.tile([P, 1], dt.float32, tag="xnn")
                act(xnn[:], b["a2"][:], AF.Sqrt)
                # bf16x2 lC rows matching rC: [xah x2, xal, ch, ch, cl];
                # packed [P,32] then one XBAR DMA transpose lands them at
                # partition base 32 to match rC's base in tbl
                lCt = smp.tile([P, 12], dt.bfloat16, tag="lCt")
                tmp3 = smp.tile([P, 3], dt.float32, tag="tmp3")
                cc1 = smp.tile([P, 1], dt.float32, tag="cc1")
                nc.vector.tensor_copy(lCt[:, 0:3], xa4[:, 0:3])
                nc.vector.tensor_copy(lCt[:, 3:6], lCt[:, 0:3])
                nc.vector.tensor_copy(tmp3[:], lCt[:, 0:3])
                nc.vector.tensor_tensor(tmp3[:], xa4[:, 0:3], tmp3[:],
                                        OP.subtract)
                nc.vector.tensor_copy(lCt[:, 6:9], tmp3[:])
                nc.vector.tensor_scalar(cc1[:], xnn[:], -0.8, None, OP.mult)
                nc.vector.tensor_copy(lCt[:, 9:10], cc1[:])
                nc.vector.tensor_copy(lCt[:, 10:11], lCt[:, 9:10])
                nc.vector.tensor_copy(tmp3[:, 0:1], lCt[:, 9:10])
                nc.vector.tensor_tensor(tmp3[:, 0:1], cc1[:], tmp3[:, 0:1],
                                        OP.subtract)
                nc.vector.tensor_copy(lCt[:, 11:12], tmp3[:, 0:1])
                # DRAM round trip: store [P,12] then load back with the
                # axes swapped; lands at partition base 32 to match rC
                xad = drp.tile([P, 12], dt.bfloat16, tag="xad")
                nc.sync.dma_start(xad[:], lCt[:])
                lCp = smp.tile([44, P], dt.bfloat16, tag="lCp")
                lC = lCp[32:44, :]
                nc.sync.dma_start(lC, xad[:].rearrange("a b -> b a"))
                b["lC"] = lC
                ssV = smp.tile([P, NPAIR], dt.float32, tag="ssV")
                sgn32 = smp.tile([P, NPAIR - len(CNT_DVE)], dt.float32,
                                 tag="sgn32")
                cntV = (smp.tile([P, len(CNT_DVE)], dt.float32, tag="cntV")
                        if CNT_DVE else None)
                b.update(ssV=ssV, sgn32=sgn32, cntV=cntV, jsg=0, jc=0)

            def emit_C_pairs(b, pj0, pj1, st=None, alt=False):
                lC, E = b["lC"], b["E"]
                for pj in range(pj0, pj1):
                    # during the exp window and the tail, psA sits idle (the
                    # last sqrt released it): alternate pools to double the
                    # psum drain depth
                    pool = psA if (alt and pj % 2 == 1) else psC
                    pC = pool.tile([P, 2 * VCH], dt.float32,
                                   tag="pA" if pool is psA else "pC")
                    for half in range(2):
                        nc.tensor.matmul(
                            pC[:, ts(half, VCH)], lC,
                            rC[:, ts(2 * pj + half, VCH)],
                            start=True, stop=True,
                        )
                    jnk = jkV.tile([P, 2 * VCH], dt.bfloat16, tag="jnkv")
                    nc.vector.scalar_tensor_tensor(
                        out=jnk[:], in0=pC[:], scalar=0.0,
                        in1=E[:, ts(pj, 2 * VCH)],
                        op0=OP.is_gt, op1=OP.mult,
                        accum_out=b["ssV"][:, pj:pj + 1],
                    )
                    if pj in CNT_DVE:
                        jnk2 = jkV.tile([P, 2 * VCH], dt.bfloat16, tag="jnkc")
                        nc.vector.tensor_scalar(
                            jnk2[:], pC[:], 0.0, None, OP.is_gt, OP.add,
                            accum_out=b["cntV"][:, b["jc"]:b["jc"] + 1],
                        )
                        b["jc"] += 1
                    else:
                        jnk2 = jkA.tile([P, 2 * VCH], dt.bfloat16, tag="jnka")
                        act(jnk2[:], pC[:], AF.Sign,
                            accum_out=b["sgn32"][:, b["jsg"]:b["jsg"] + 1])
                        b["jsg"] += 1

            def emit_D(b):
                i = b["i"]
                sst = smp.tile([P, 1], dt.float32, tag="sst")
                sgs = smp.tile([P, 1], dt.float32, tag="sgs")
                nc.vector.reduce_sum(sst[:], b["ssV"][:],
                                     axis=mybir.AxisListType.X)
                nc.vector.reduce_sum(sgs[:], b["sgn32"][:],
                                     axis=mybir.AxisListType.X)
                cntt = smp.tile([P, 1], dt.float32, tag="cntt")
                # cnt = cnt_dve + (sum_sign + 1024*n_sign_pairs)/2
                nc.vector.tensor_scalar(
                    cntt[:], sgs[:], 0.5,
                    float(VCH * (NPAIR - len(CNT_DVE))),
                    OP.mult, OP.add,
                )
                if b["cntV"] is not None:
                    cnv = smp.tile([P, 1], dt.float32, tag="cnv")
                    nc.vector.reduce_sum(cnv[:], b["cntV"][:],
                                         axis=mybir.AxisListType.X)
                    nc.vector.tensor_tensor(cntt[:], cntt[:], cnv[:], OP.add)
                nz = smp.tile([P, 1], dt.float32, tag="nz")
                nc.vector.tensor_scalar(nz[:], cntt[:], 0.5, None, OP.is_gt)
                cc = smp.tile([P, 1], dt.float32, tag="cc")
                nc.vector.tensor_scalar(cc[:], cntt[:], 1.0, None, OP.max)
                rec = smp.tile([P, 1], dt.float32, tag="rec")
                nc.vector.reciprocal(rec[:], cc[:])
                fld = smp.tile([P, 1], dt.float32, tag="fld")
                nc.vector.tensor_tensor(fld[:], sst[:], rec[:], OP.mult)
                nc.vector.tensor_tensor(fld[:], fld[:], nz[:], OP.mult)
                nc.sync.dma_start(of_d[ts(i, P)], fld[:])
                nc.sync.dma_start(on_d[ts(i, P)], nz[:])

            prev = None
            for i in range(NT):
                a = emit_A(i, prev)
                if prev is not None:
                    emit_D(prev)
                prev = emit_B(a)
            emit_finishB(prev)
            emit_C_pairs(prev, 0, NPAIR, alt=True)
            emit_D(prev)

    if hw:
        _split_multiwaits(nc)
    return nc


def _split_multiwaits(nc):
    """This toolchain's walrus accepts at most ONE sync wait per
    instruction (setupSyncWait<...> hard-errors otherwise). Tile attaches
    all required waits to the consuming instruction, so split every
    extra wait into a standalone EventSemaphore on the same engine queue
    right before the instruction (the raw-Bass wait_ge pattern)."""
    import concourse.mybir as mybir

    n = 0
    for bb in nc.main_func.blocks:
        insts = bb.instructions
        out = []
        for inst in insts:
            si = inst.sync_info
            if si is not None and len(si.on_wait) > 1:
                waits = list(si.on_wait)
                for w in waits[:-1]:
                    ev = mybir.InstEventSemaphore(name=f"W-split-{n}")
                    n += 1
                    ev.engine = inst.engine
                    ev.debug = inst.debug
                    ev.sync_info = mybir.SyncInfo(on_wait=[w], on_update=[])
                    out.append(ev)
                inst.sync_info = mybir.SyncInfo(
                    on_wait=[waits[-1]], on_update=list(si.on_update)
                )
            out.append(inst)
        bb.instructions = out


def _prep_inputs(x_world, voxel_point, voxel_normal, score, d_a, d_b):
    """Host-side prep: per-core in_maps for the SPMD program."""
    x = np.ascontiguousarray(x_world[:, 0, :], dtype=F32)          # [N,3]
    p = np.ascontiguousarray(voxel_point[0, :, :3], dtype=F32)     # [V,3]
    nrm = np.ascontiguousarray(voxel_normal, dtype=F32)            # [V,3]
    sc = np.asarray(score, dtype=F32)
    da = float(np.asarray(d_a).reshape(-1)[0])
    db = float(np.asarray(d_b).reshape(-1)[0])

    def s2(a):
        """bf16x2 split: hi + lo as float32."""
        h = a.astype(BF).astype(F32)
        return h, (a - h).astype(F32)

    # tbl rows 0-12 (rA): per coord [ph, pl, ph], then [1, 1, p2h, p2l]
    # pairing lA rows [xh, xh, xl]*3, [x2h, x2l, 1, 1]:
    #   psum = 2x.p - x2 - p2 = -(dist^2) to ~2e-5
    # tbl rows 32-43 (rC): [nh(3), nl(3), nh(3), bh, bl, bh]
    p2h, p2l = s2(-(p * p).sum(1, dtype=F32))
    b = np.sqrt((nrm * nrm).sum(1, dtype=F32)).astype(F32)
    nh, nl = s2(nrm)
    bh, bl = s2(b)
    tbl = np.zeros((44, V), F32)
    for i in range(3):
        ph, pl = s2(p[:, i])
        tbl[3 * i + 0] = ph
        tbl[3 * i + 1] = pl
        tbl[3 * i + 2] = ph
    tbl[9] = 1.0
    tbl[10] = 1.0
    tbl[11] = p2h
    tbl[12] = p2l
    tbl[32:35] = nh.T
    tbl[35:38] = nl.T
    tbl[38:41] = nh.T
    tbl[41] = bh
    tbl[42] = bl
    tbl[43] = bh
    tbl = tbl.astype(BF)

    pk = np.zeros((V, PKW), F32)
    pk[:, 0:3] = p
    pk[:, 3:6] = nrm
    pk16 = np.ascontiguousarray(pk.reshape(NSEG, SEGW * PKW))
    scp = (sc * (1.0 / da)).astype(F32).astype(BF)

    in_maps = []
    for cid in range(N_CORES):
        sl = slice(cid * NQ, (cid + 1) * NQ)
        xc = x[sl]                                                  # [NQ,3]
        x2h, x2l = s2(-(xc * xc).sum(1, dtype=F32))
        lA = np.zeros((13, NQ), F32)
        for i in range(3):
            xh, xl = s2(2.0 * xc[:, i])
            lA[3 * i + 0] = xh
            lA[3 * i + 1] = xh
            lA[3 * i + 2] = xl
        lA[9] = x2h
        lA[10] = x2l
        lA[11] = 1.0
        lA[12] = 1.0
        in_maps.append({
            "lA": lA.astype(BF), "tbl": tbl, "pk": pk16, "scp": scp,
            "xq": xc,
        })
    return in_maps, db


def _get_runner(nc):
    """Build (once) a jitted 8-core SPMD runner for the program."""
    import jax
    from jax.sharding import Mesh, PartitionSpec, NamedSharding
    try:
        from jax.experimental.shard_map import shard_map
    except Exception:
        from jax.shard_map import shard_map
    from concourse import bass2jax
    import concourse.mybir as mybir

    bass2jax.install_neuronx_cc_hook()
    pname = nc.partition_id_tensor.name if nc.partition_id_tensor else None
    in_names, out_names, out_avals, zero_outs = [], [], [], []
    for alloc in nc.m.functions[0].allocations:
        if not isinstance(alloc, mybir.MemoryLocationSet):
            continue
        name = alloc.memorylocations[0].name
        if alloc.kind == "ExternalInput":
            if name != pname:
                in_names.append(name)
        elif alloc.kind == "ExternalOutput":
            shape = tuple(alloc.tensor_shape)
            dtype = mybir.dt.np(alloc.dtype)
            out_names.append(name)
            out_avals.append(jax.core.ShapedArray(shape, dtype))
            zero_outs.append(np.zeros(shape, dtype))
    all_names = list(in_names) + list(out_names) + ([pname] if pname else [])

    def _body(*args):
        operands = list(args)
        if pname:
            operands.append(bass2jax.partition_id_tensor())
        return tuple(bass2jax._bass_exec_p.bind(
            *operands, out_avals=tuple(out_avals), in_names=tuple(all_names),
            out_names=tuple(out_names), lowering_input_output_aliases=(),
            sim_require_finite=True, sim_require_nnan=True, nc=nc))

    devices = jax.devices()[:N_CORES]
    mesh = Mesh(np.asarray(devices), ("core",))
    nin = len(in_names) + len(out_names)
    fn = jax.jit(shard_map(
        _body, mesh=mesh, in_specs=(PartitionSpec("core"),) * nin,
        out_specs=(PartitionSpec("core"),) * len(out_names),
        check_rep=False), keep_unused=True)
    sharding = NamedSharding(mesh, PartitionSpec("core"))

    def run(in_maps):
        concat = [np.concatenate([np.asarray(in_maps[c][nm])
                                  for c in range(N_CORES)], axis=0)
                  for nm in in_names]
        concat += [np.concatenate([z] * N_CORES, axis=0) for z in zero_outs]
        import jax as _j
        dev = [_j.device_put(a, sharding) for a in concat]
        outs = fn(*dev)
        o = {nm: np.asarray(outs[i]) for i, nm in enumerate(out_names)}
        return o

    return run


def kernel(**inputs):
    in_maps, db = _prep_inputs(
        inputs["x_world"], inputs["voxel_point"], inputs["voxel_normal"],
        inputs["score"], inputs["d_a"], inputs["d_b"],
    )
    key = ("prog", db)
    if key not in _prog_cache:
        _prog_cache[key] = _build_program(-db)
    nc = _prog_cache[key]

    try:
        rkey = ("runner", db)
        if rkey not in _prog_cache:
            _prog_cache[rkey] = _get_runner(nc)
        o = _prog_cache[rkey](in_maps)
        field = o["of"].reshape(N_CORES, NQ).reshape(-1)
        nzf = o["on"].reshape(-1)
    except Exception:
        from concourse.bass_utils import run_bass_kernel_spmd
        res = run_bass_kernel_spmd(nc, in_maps, list(range(N_CORES))).results
        field = np.concatenate([np.asarray(r["of"]).reshape(-1) for r in res])
        nzf = np.concatenate([np.asarray(r["on"]).reshape(-1) for r in res])
    return field.astype(F32), (nzf > 0.5)
